# revision 39
# baseline (speedup 1.0000x reference)
"""Bass/Trainium2 kernel for a 12-head self-attention block
(B=8, T=1024, C=768), data-parallel across 8 NeuronCores (one batch
element per core).

Per-core computation (batch element b):
  qkv   = x @ W_attn + b_attn            [T, 3C]
  scoresT[k, q] = k_h . q_h / 8 (+ mask bias), keys on partitions
  e     = exp(scoresT) (unnormalized; denominator accumulated via a
          ones-column appended to v in the AV matmul)
  out_h = (v_ext.T @ e) / denom-row
  y     = concat(out_h) @ W_proj + b_proj

Layout scheme (no on-device transposes anywhere):
  - host passes xT = x[b].T                       [C, T]
  - qT/kT computed as  qkT[c', t] = W_attn[:, :1536].T @ x.T
  - v computed as       v[t, c'] = x @ W_attn[:, 1536:]
  - scoresT[k, q] = kT_h.T @ qT_h; head pairs (2j, 2j+1) at partition
    offsets 0/64 share one [128, 1024] PSUM tile and one Exp ACTIVATE
  - AV: out_extT[d_ext, q] = v_ext.T @ expT, v_ext = [v_h | 1]
    row 64 of the 65-row result is the softmax denominator
  - normalization: denominator row -> DRAM-bounce partition broadcast ->
    reciprocal_approx_fast -> one DVE multiply into the dead qT half
  - projection: y[t, c2] = concatT.T @ W_proj, bias seeded via ones-matmul

Scheduling: the attention inner loop alone is Act-bound (exp of
[128, 1024] takes ~1.11 us while its 4 matmuls need only ~0.85 us at full
clock), and any PE idle gap resets the PE DVFS ramp (2.4 GHz requires
~3 us of continuous execution; idling drops it to 1.2 GHz). So instead of
serial QKV / attention / proj phases, the v / qk-prefetch / projection
chunks are generators yielding one PSUM-accumulation bundle (>=2 matmuls,
~850-1000 columns) per pop, and every attention iteration pumps exactly
one bundle into the exp-wait slack (262 ns needed, ~427 ns provided —
strictly every iteration, since even one short iteration reseeds the
half-clock oscillation). Pair 0's first sweep is fed by draining the v
chunks just-in-time before each AV; the qk prefetches feed pairs 1-5;
the first-half projection chunks (legal once every pair has written its
qc0 concat) feed pair 5's second sweep; the rest of the projection
drains in the epilogue with its PSUM->SBUF copies on the then-idle Act
engine. Chunks hold ONE fill-PSUM slot at a time so two chunks pipeline.
The AV accumulators are evacuated to SBUF immediately after the last AV
so the next pair never waits on the ~5 us DRAM-bounce normalization.
PSUM: 2x[128,1024] score tiles + 2x[65,512] AV accumulators + 2x[128,512]
fill accumulators = 8 banks.

Dtypes: x / W_attn are bf16 (halves the input DMA; matmul rate is
unchanged at 1 col/cycle), everything internal stays float32r (full rate
at free-dim >= 256); PSUM accumulation is fp32. Measured ~230 us/core vs
the 247 us serial-phase baseline; rel err ~2.5e-3 (gate 2e-2).
"""

import sys

if "/opt/trn_rl_repo" not in sys.path:
    sys.path.insert(0, "/opt/trn_rl_repo")

from collections import deque
from contextlib import ExitStack

import ml_dtypes
import numpy as np

import concourse.bass as bass
import concourse.tile as tile
from concourse import bacc, mybir
from concourse import bass_utils

N_HEAD = 12
B = 8
T = 1024
C = 768
HD = 64
KO = C // 128          # 6 contraction chunks of 128
TC = T // 128          # 8 token chunks of 128
QN = T // 512          # 2 query chunks of 512
NPAIR = N_HEAD // 2    # 6 head pairs

F32 = mybir.dt.float32
F32R = mybir.dt.float32r
BF16 = mybir.dt.bfloat16
AF = mybir.ActivationFunctionType

_cache: dict = {}
_ONES = np.ones((128, 128), np.float32)


def _emit_kernel(tc_ctx, aps):
    nc = tc_ctx.nc
    ctx = aps["ctx"]
    xT_d, wa_d, wp_d, bqk_d, bv_d, bp_d, mb_d, y_d, ones_d = (
        aps["xT"], aps["Wa"], aps["Wp"], aps["bqk"], aps["bv"], aps["bp"],
        aps["mb"], aps["y"], aps["ones"],
    )

    const = ctx.enter_context(tc_ctx.tile_pool(name="const", bufs=1))
    wqk_pool = ctx.enter_context(tc_ctx.tile_pool(name="wqk", bufs=6))
    e_pool = ctx.enter_context(tc_ctx.tile_pool(name="e", bufs=4))
    r_pool = ctx.enter_context(tc_ctx.tile_pool(name="r", bufs=2))
    rb_pool = ctx.enter_context(tc_ctx.tile_pool(name="rb", bufs=2))
    tmp_pool = ctx.enter_context(tc_ctx.tile_pool(name="tmp", bufs=3))
    rd_pool = ctx.enter_context(tc_ctx.tile_pool(name="rd", bufs=3, space="DRAM"))
    out_pool = ctx.enter_context(tc_ctx.tile_pool(name="out", bufs=4))

    # PSUM: 8 banks = 2x[128,1024] scores + 2x[65,512] AV + 2x[128,512] fill
    sc_ps = ctx.enter_context(tc_ctx.tile_pool(name="scps", bufs=2, space="PSUM"))
    av_ps = ctx.enter_context(tc_ctx.tile_pool(name="avps", bufs=2, space="PSUM"))
    fl_ps = ctx.enter_context(tc_ctx.tile_pool(name="flps", bufs=2, space="PSUM"))

    # ---- persistent SBUF tensors -------------------------------------
    xT_sb = [const.tile([128, T], BF16, name=f"xT_{ko}") for ko in range(KO)]
    wv_sb = [const.tile([128, C], BF16, name=f"wv_{ko}") for ko in range(KO)]
    wp_sb = const.tile([128, KO, C], F32R)       # W_proj
    # per head-pair j: [:, 0, :] = qT chunk j (later overwritten by the
    # pair's normalized concat output), [:, 1, :] = kT chunk 6+j
    qk_sb = [const.tile([128, 2, T], F32R, name=f"qk_{j}") for j in range(NPAIR)]
    v_sb = const.tile([128, TC, N_HEAD, HD + 1], F32R)  # +1 = ones column
    bqk_sb = const.tile([128, 12], F32)
    mb_sb = const.tile([128, TC], F32)
    bv_sb = const.tile([1, C], F32R)
    bp_sb = const.tile([1, C], F32R)
    ones_sb = const.tile([128, 128], F32R)
    wqk_sb: dict = {}

    def prefetch_wqk(m, eng=None, split=False):
        # 6-slot ring: the DMA into a reused slot waits (in-queue) for the
        # previous chunk's matmuls, which throttles prefetch automatically.
        # split=True lands ko chunks individually so the first ldweights can
        # start as soon as possible (prologue-critical chunks only).
        t = wqk_pool.tile([128, KO, 128], BF16, tag="wqk", name=f"wqk_{m}")
        if split:
            for ko in range(KO):
                (eng or nc.sync).dma_start(t[:, ko], wqk_r[m][:, ko])
        else:
            (eng or nc.sync).dma_start(t[:], wqk_r[m])
        wqk_sb[m] = t

    # ---- input DMAs --------------------------------------------------
    # sync queue: x only (the compute-critical stream); vector queue: W_v;
    # gpsimd queue: small tensors first, the fat W_proj last. The v ones
    # column is memset on-engine (a strided DMA here took ~30us and jammed
    # the queue behind it).
    xT_r = xT_d.rearrange("(ko p) t -> p ko t", p=128)
    wv_r = wa_d[:, 2 * C : 3 * C].rearrange("(ko p) n -> p ko n", p=128)
    wqk_r = [
        wa_d[:, m * 128 : (m + 1) * 128].rearrange("(ko p) n -> p ko n", p=128)
        for m in range(12)
    ]
    prefetch_wqk(0)
    nc.scalar.dma_start(ones_sb[:], ones_d)
    nc.scalar.dma_start(bv_sb[:], bv_d)
    nc.scalar.dma_start(bqk_sb[:], bqk_d)
    nc.scalar.dma_start(mb_sb[:], mb_d)
    nc.scalar.dma_start(bp_sb[:], bp_d)
    for ko in range(KO):
        nc.sync.dma_start(xT_sb[ko][:], xT_r[:, ko])
        if ko == 0:
            prefetch_wqk(6)
        nc.scalar.dma_start(wv_sb[ko][:], wv_r[:, ko])
    # remaining qk weights, in consumption order
    for j in range(1, NPAIR):
        prefetch_wqk(j)
        prefetch_wqk(6 + j)
    nc.scalar.dma_start(wp_sb[:], wp_d.rearrange("(ko p) n -> p ko n", p=128))
    nc.vector.tensor_copy(
        out=v_sb[:, :, :, HD],
        in_=ones_sb[:, 0 : TC * N_HEAD].rearrange("p (a b) -> p a b", b=N_HEAD),
    )
    # warm the Exp activation table while the prologue DMAs stream
    warm_sb = const.tile([1, 2], F32, name="warm")
    nc.vector.memset(warm_sb[:], 0.0)
    nc.scalar.activation(warm_sb[0:1, 0:1], warm_sb[0:1, 1:2], AF.Exp)

    segs = [(0, 512), (512, 256)]

    # ---- fill-chunk generators (yield once per PE bundle) ------------
    def gen_v_chunk(tcc):
        # one PSUM tile at a time so two chunks can pipeline in the 2-slot
        # fill pool (a chunk holding both slots serializes chunk boundaries)
        for j, (off, w) in enumerate(segs):
            ps = fl_ps.tile([128, 512], F32, tag="fl", name=f"ps1b_{tcc}_{j}")
            nc.tensor.matmul(
                ps[:, :w],
                ones_sb[0:1, 0:128],
                bv_sb[0:1, off : off + w],
                start=True,
                stop=False,
            )
            for ko in range(KO):
                nc.tensor.matmul(
                    ps[:, :w],
                    xT_sb[ko][:, tcc * 128 : (tcc + 1) * 128],
                    wv_sb[ko][:, off : off + w],
                    start=False,
                    stop=(ko == KO - 1),
                )
                if ko in (1, 3):
                    yield
            nc.vector.tensor_copy(
                out=v_sb[:, tcc, off // HD : (off + w) // HD, 0:HD],
                in_=ps[:, :w].rearrange("p (h d) -> p h d", d=HD),
            )
            if j == 0:
                yield

    def gen_qk_chunk(j, half, m):
        # half 0 -> qT chunk (m = j), half 1 -> kT chunk (m = 6 + j)
        wqk = wqk_sb[m]
        pss = [
            fl_ps.tile([128, 512], F32, tag="fl", name=f"ps1a_{m}_{i}")
            for i in range(QN)
        ]
        for ko in range(KO):
            for nq in range(QN):
                nc.tensor.matmul(
                    pss[nq],
                    wqk[:, ko, :],
                    xT_sb[ko][:, nq * 512 : (nq + 1) * 512],
                    start=(ko == 0),
                    stop=(ko == KO - 1),
                )
            if ko < KO - 1:
                yield
        # psum -> SBUF with per-partition bias add (b_attn) on DVE
        for nq in range(QN):
            nc.vector.tensor_tensor(
                qk_sb[j][:, half, nq * 512 : (nq + 1) * 512],
                pss[nq],
                bqk_sb[:, m : m + 1].to_broadcast((128, 512)),
                mybir.AluOpType.add,
            )

    def gen_proj_chunk(tcc):
        o_sb = out_pool.tile([128, C], F32, tag="out", name=f"o_{tcc}")
        for j, (off, w) in enumerate(segs):
            ps = fl_ps.tile([128, 512], F32, tag="fl", name=f"ps4_{tcc}_{j}")
            nc.tensor.matmul(
                ps[:, :w],
                ones_sb[0:1, 0:128],
                bp_sb[0:1, off : off + w],
                start=True,
                stop=False,
            )
            for ko in range(KO):
                nc.tensor.matmul(
                    ps[:, :w],
                    qk_sb[ko][:, 0, tcc * 128 : (tcc + 1) * 128],
                    wp_sb[:, ko, off : off + w],
                    start=False,
                    stop=(ko == KO - 1),
                )
                if ko in (1, 3):
                    yield
            nc.scalar.activation(o_sb[:, off : off + w], ps[:, :w], AF.Copy)
            if j == 0:
                yield
        nc.sync.dma_start(y_d[tcc * 128 : (tcc + 1) * 128, :], o_sb[:])

    # ---- fill queue machinery ----------------------------------------
    fills: deque = deque()
    done: dict = {}

    def add_fill(name, gen):
        def wrapped():
            yield from gen
            done[name] = True
        done[name] = False
        fills.append(wrapped())

    def pump(n=1):
        k = 0
        while fills and k < n:
            try:
                next(fills[0])
                k += 1
            except StopIteration:
                fills.popleft()

    def drain(name):
        while not done[name]:
            if not fills:
                raise RuntimeError(f"drain({name}): queue empty")
            try:
                next(fills[0])
            except StopIteration:
                fills.popleft()

    # ---- attention for one head pair ---------------------------------
    pending_norm = [None]

    def emit_pair_attention(j, qc, pump_n=None):
        if True:
            qk = qk_sb[j]
            qsl = slice(qc * 512, (qc + 1) * 512)
            ava = av_ps.tile([65, 512], F32, tag="av", name=f"ava_{j}_{qc}")
            avb = av_ps.tile([65, 512], F32, tag="av", name=f"avb_{j}_{qc}")

            def emit_av(kc, e):
                if j == 0:
                    drain(f"v{kc}")
                nc.tensor.matmul(
                    ava, v_sb[:, kc, 2 * j, :], e[:, 0:512],
                    start=(kc == 0), stop=(kc == TC - 1),
                )
                nc.tensor.matmul(
                    avb, v_sb[:, kc, 2 * j + 1, :], e[:, 512:1024],
                    start=(kc == 0), stop=(kc == TC - 1),
                )

            prev = None
            for kc in range(TC):
                ksl = slice(kc * 128, (kc + 1) * 128)
                sc = sc_ps.tile([128, 1024], F32, tag="sc", name=f"sc_{j}_{qc}_{kc}")
                # head a (partitions 0-63) and head b (64-127)
                nc.tensor.matmul(
                    sc[:, 0:512], qk[0:64, 1, ksl], qk[0:64, 0, qsl],
                    start=True, stop=True,
                )
                nc.tensor.matmul(
                    sc[:, 512:1024], qk[64:128, 1, ksl], qk[64:128, 0, qsl],
                    start=True, stop=True,
                )
                e = e_pool.tile([128, 1024], F32R, tag="e", name=f"e_{j}_{qc}_{kc}")
                nc.scalar.activation(
                    e, sc, AF.Exp, bias=mb_sb[:, kc : kc + 1], scale=0.125
                )
                # issue the PREVIOUS iteration's AV matmuls after the next
                # score matmuls, with one fill bundle covering the exp wait
                if kc == 1 and pending_norm[0] is not None:
                    pending_norm[0]()
                    pending_norm[0] = None
                if prev is not None:
                    if not (j == 0 and qc == 0):
                        pump(pump_n if pump_n is not None else 1)
                    emit_av(*prev)
                prev = (kc, e)
            emit_av(*prev)

            # normalize — emission DEFERRED until after the NEXT sweep's
            # first scores: the concat-shift DMA below is tracked at tile
            # granularity, so emitting it before the next sweep's reads of
            # qk_sb would make those scores falsely wait out the whole
            # ~2-3us DRAM-bounce chain at every sweep boundary.
            def do_norm(j=j, qc=qc, qk=qk, qsl=qsl, ava=ava, avb=avb):
                # evacuate the AV accumulators first so the PSUM slots free
                r_t = r_pool.tile([65, 1024], F32, tag="r", name=f"r_{j}_{qc}")
                t_sb = tmp_pool.tile([64, 512], F32R, tag="tmp", name=f"tmp_{j}_{qc}")
                nc.vector.tensor_copy(out=r_t[64:65, 0:512], in_=ava[64:65, :])
                nc.vector.tensor_copy(out=r_t[64:65, 512:1024], in_=avb[64:65, :])
                nc.vector.tensor_copy(out=qk[0:64, 0, qsl], in_=ava[0:64, :])
                nc.vector.tensor_copy(out=t_sb[:], in_=avb[0:64, :])
                # denominators -> DRAM-bounce partition broadcast -> fast
                # reciprocal -> multiply in place
                rd = rd_pool.tile([1, 1024], F32, tag="rd", name=f"rd_{j}_{qc}")
                nc.gpsimd.dma_start(rd[:], r_t[64:65, :])
                rbw = rb_pool.tile([64, 1024], F32, tag="rbw", name=f"rbw_{j}_{qc}")
                nc.gpsimd.dma_start(rbw[:], rd.to_broadcast((64, 1024)))
                rb = rb_pool.tile([64, 1024], F32, tag="rb", name=f"rb_{j}_{qc}")
                nc.vector.reciprocal_approx_fast(out=rb[:], in_=rbw[:])
                # head a -> concat partitions 0-63 (in place in the qT half)
                nc.vector.tensor_mul(
                    out=qk[0:64, 0, qsl], in0=qk[0:64, 0, qsl], in1=rb[:, 0:512]
                )
                # head b -> concat partitions 64-127 (SBUF->SBUF DMA shift)
                nc.vector.tensor_mul(out=t_sb[:], in0=t_sb[:], in1=rb[:, 512:1024])
                nc.gpsimd.dma_start(qk[64:128, 0, qsl], t_sb[:])

            pending_norm[0] = do_norm

    # ---- schedule (qc-major) -----------------------------------------
    # prologue: pair 0's q/k chunks inline, interleaved per-ko so both
    # consume each xT chunk as its DMA lands; everything else is fill work.
    # qc-major order lets the first-half projection chunks overlap the
    # whole second attention sweep instead of forming a serial tail.
    alive = [gen_qk_chunk(0, 0, 0), gen_qk_chunk(0, 1, 6)]
    while alive:
        for gen in list(alive):
            try:
                next(gen)
            except StopIteration:
                alive.remove(gen)
    for tcc in range(TC):
        add_fill(f"v{tcc}", gen_v_chunk(tcc))
    for j in range(1, NPAIR):
        add_fill(f"q{j}", gen_qk_chunk(j, 0, j))
        add_fill(f"k{j}", gen_qk_chunk(j, 1, 6 + j))

    # j-major sweep with a precisely rationed fill diet: gapless PE needs
    # ~262ns of fill per iteration (Act exp 1115ns minus scores+AV 853ns);
    # alternating pump(1)/pump(2) of 512-col half-bundles averages ~320ns,
    # making the qk prefetch supply last through all six pairs.
    for j in range(NPAIR):
        if j > 0:
            drain(f"q{j}")
            drain(f"k{j}")
        emit_pair_attention(j, 0)
        if j == NPAIR - 1:
            # every pair has written its first-half concat: the first four
            # projection chunks can fill pair 5's second sweep
            for tcc in range(4):
                add_fill(f"p{tcc}", gen_proj_chunk(tcc))
        emit_pair_attention(j, 1, pump_n=2 if j == NPAIR - 1 else None)

    # drain the leftover first-half projection pops BEFORE emitting the
    # last sweep's norm — emitted after it they would falsely wait out the
    # whole DRAM-bounce chain (tile-granular DMA tracking); emitted before,
    # they cover the chain's ~5us latency with real PE work
    while fills:
        pump(4)
    # the last sweep's norm must precede the second-half projection (its
    # ko=5 matmuls genuinely consume pair 5's qc1 concat)
    pending_norm[0]()
    pending_norm[0] = None
    for tcc in range(4, TC):
        add_fill(f"p{tcc}", gen_proj_chunk(tcc))
    while fills:
        pump(4)


def _get_program():
    if "nc" in _cache:
        return _cache["nc"]
    nc = bacc.Bacc(
        "TRN2", target_bir_lowering=False, debug=False, enable_asserts=True
    )
    aps = {
        "xT": nc.dram_tensor("xT", [C, T], BF16, kind="ExternalInput").ap(),
        "Wa": nc.dram_tensor("Wa", [C, 3 * C], BF16, kind="ExternalInput").ap(),
        "Wp": nc.dram_tensor("Wp", [C, C], F32R, kind="ExternalInput").ap(),
        "bqk": nc.dram_tensor("bqk", [128, 12], F32, kind="ExternalInput").ap(),
        "bv": nc.dram_tensor("bv", [1, C], F32R, kind="ExternalInput").ap(),
        "bp": nc.dram_tensor("bp", [1, C], F32R, kind="ExternalInput").ap(),
        "mb": nc.dram_tensor("mb", [128, TC], F32, kind="ExternalInput").ap(),
        "y": nc.dram_tensor("y", [T, C], F32, kind="ExternalOutput").ap(),
        "ones": nc.dram_tensor("ones", [128, 128], F32R, kind="ExternalInput").ap(),
    }
    with tile.TileContext(nc) as tc_ctx, ExitStack() as ctx:
        aps["ctx"] = ctx
        _emit_kernel(tc_ctx, aps)
    nc.compile()
    _cache["nc"] = nc
    return nc


def _make_in_maps(inputs):
    x = np.asarray(inputs["x"], np.float32)
    mask = np.asarray(inputs["attn_mask"])
    Wa = np.asarray(inputs["W_attn"], np.float32)
    ba = np.asarray(inputs["b_attn"], np.float32)
    Wp = np.asarray(inputs["W_proj"], np.float32)
    bp = np.asarray(inputs["b_proj"], np.float32)

    bqk = np.ascontiguousarray(ba[: 2 * C].reshape(12, 128).T)
    bv = np.ascontiguousarray(ba[2 * C :].reshape(1, C))
    bpr = np.ascontiguousarray(bp.reshape(1, C))
    Wab = np.ascontiguousarray(Wa.astype(ml_dtypes.bfloat16))
    Wpb = np.ascontiguousarray(Wp)
    in_maps = []
    for b in range(B):
        mb = np.where(mask[b] == 0, np.float32(-30.0), np.float32(0.0))
        mb = np.ascontiguousarray(mb.astype(np.float32).reshape(TC, 128).T)
        in_maps.append(
            {
                "xT": np.ascontiguousarray(x[b].T.astype(ml_dtypes.bfloat16)),
                "Wa": Wab,
                "Wp": Wpb,
                "bqk": bqk,
                "bv": bv,
                "bp": bpr,
                "mb": mb,
                "ones": _ONES,
            }
        )
    return in_maps


def _run(inputs, trace=False):
    nc = _get_program()
    in_maps = _make_in_maps(inputs)
    res = bass_utils.run_bass_kernel_spmd(
        nc, in_maps, core_ids=list(range(B)), trace=trace
    )
    y = np.stack([res.results[b]["y"] for b in range(B)], axis=0)
    return y, res


def kernel(**inputs) -> np.ndarray:
    y, _ = _run(inputs, trace=False)
    return y


# revision 40
# speedup vs baseline: 1.0227x; 1.0227x over previous
"""Bass/Trainium2 kernel for a 12-head self-attention block
(B=8, T=1024, C=768), data-parallel across 8 NeuronCores (one batch
element per core).

Per-core computation (batch element b):
  qkv   = x @ W_attn + b_attn            [T, 3C]
  scoresT[k, q] = k_h . q_h / 8 (+ mask bias), keys on partitions
  e     = exp(scoresT) (unnormalized; denominator accumulated via a
          ones-column appended to v in the AV matmul)
  out_h = (v_ext.T @ e) / denom-row
  y     = concat(out_h) @ W_proj + b_proj

Layout scheme (no on-device transposes anywhere):
  - host passes xT = x[b].T                       [C, T]
  - qT/kT computed as  qkT[c', t] = W_attn[:, :1536].T @ x.T
  - v computed as       v[t, c'] = x @ W_attn[:, 1536:]
  - scoresT[k, q] = kT_h.T @ qT_h; head pairs (2j, 2j+1) at partition
    offsets 0/64 share one [128, 1024] PSUM tile and one Exp ACTIVATE
  - AV: out_extT[d_ext, q] = v_ext.T @ expT, v_ext = [v_h | 1]
    row 64 of the 65-row result is the softmax denominator
  - normalization: denominator row -> DRAM-bounce partition broadcast ->
    reciprocal_approx_fast -> one DVE multiply into the dead qT half
  - projection: y[t, c2] = concatT.T @ W_proj, bias seeded via ones-matmul

Scheduling: the attention inner loop alone is Act-bound (exp of
[128, 1024] takes ~1.11 us while its 4 matmuls need only ~0.85 us at full
clock), and any PE idle gap resets the PE DVFS ramp (2.4 GHz requires
~3 us of continuous execution; idling drops it to 1.2 GHz). So instead of
serial QKV / attention / proj phases, the v / qk-prefetch / projection
chunks are generators yielding one PSUM-accumulation bundle (>=2 matmuls,
~850-1000 columns) per pop, and every attention iteration pumps exactly
one bundle into the exp-wait slack (262 ns needed, ~427 ns provided —
strictly every iteration, since even one short iteration reseeds the
half-clock oscillation). Pair 0's first sweep is fed by draining the v
chunks just-in-time before each AV; the qk prefetches feed pairs 1-5;
the first-half projection chunks (legal once every pair has written its
qc0 concat) feed pair 5's second sweep; the rest of the projection
drains in the epilogue with its PSUM->SBUF copies on the then-idle Act
engine. Chunks hold ONE fill-PSUM slot at a time so two chunks pipeline.
The AV accumulators are evacuated to SBUF immediately after the last AV
so the next pair never waits on the ~5 us DRAM-bounce normalization.
PSUM: 2x[128,1024] score tiles + 2x[65,512] AV accumulators + 2x[128,512]
fill accumulators = 8 banks.

Dtypes: x / W_attn are bf16 (halves the input DMA; matmul rate is
unchanged at 1 col/cycle), everything internal stays float32r (full rate
at free-dim >= 256); PSUM accumulation is fp32. Measured ~230 us/core vs
the 247 us serial-phase baseline; rel err ~2.5e-3 (gate 2e-2).
"""

import sys

if "/opt/trn_rl_repo" not in sys.path:
    sys.path.insert(0, "/opt/trn_rl_repo")

from collections import deque
from contextlib import ExitStack

import ml_dtypes
import numpy as np

import concourse.bass as bass
import concourse.tile as tile
from concourse import bacc, mybir
from concourse import bass_utils

N_HEAD = 12
B = 8
T = 1024
C = 768
HD = 64
KO = C // 128          # 6 contraction chunks of 128
TC = T // 128          # 8 token chunks of 128
QN = T // 512          # 2 query chunks of 512
NPAIR = N_HEAD // 2    # 6 head pairs

F32 = mybir.dt.float32
F32R = mybir.dt.float32r
BF16 = mybir.dt.bfloat16
AF = mybir.ActivationFunctionType

_cache: dict = {}
_ONES = np.ones((128, 128), np.float32)


def _emit_kernel(tc_ctx, aps):
    nc = tc_ctx.nc
    ctx = aps["ctx"]
    xT_d, wa_d, wp_d, bqk_d, bv_d, bp_d, mb_d, y_d, ones_d = (
        aps["xT"], aps["Wa"], aps["Wp"], aps["bqk"], aps["bv"], aps["bp"],
        aps["mb"], aps["y"], aps["ones"],
    )

    const = ctx.enter_context(tc_ctx.tile_pool(name="const", bufs=1))
    wqk_pool = ctx.enter_context(tc_ctx.tile_pool(name="wqk", bufs=6))
    e_pool = ctx.enter_context(tc_ctx.tile_pool(name="e", bufs=4))
    r_pool = ctx.enter_context(tc_ctx.tile_pool(name="r", bufs=2))
    rb_pool = ctx.enter_context(tc_ctx.tile_pool(name="rb", bufs=2))
    tmp_pool = ctx.enter_context(tc_ctx.tile_pool(name="tmp", bufs=3))
    rd_pool = ctx.enter_context(tc_ctx.tile_pool(name="rd", bufs=3, space="DRAM"))
    out_pool = ctx.enter_context(tc_ctx.tile_pool(name="out", bufs=4))

    # PSUM: 8 banks = 2x[128,1024] scores + 2x[65,512] AV + 2x[128,512] fill
    sc_ps = ctx.enter_context(tc_ctx.tile_pool(name="scps", bufs=2, space="PSUM"))
    av_ps = ctx.enter_context(tc_ctx.tile_pool(name="avps", bufs=2, space="PSUM"))
    fl_ps = ctx.enter_context(tc_ctx.tile_pool(name="flps", bufs=2, space="PSUM"))

    # ---- persistent SBUF tensors -------------------------------------
    xT_sb = [const.tile([128, T], BF16, name=f"xT_{ko}") for ko in range(KO)]
    wv_sb = [const.tile([128, C], BF16, name=f"wv_{ko}") for ko in range(KO)]
    wp_sb = const.tile([128, KO, C], F32R)       # W_proj
    # per head-pair j: [:, 0, :] = qT chunk j (later overwritten by the
    # pair's normalized concat output), [:, 1, :] = kT chunk 6+j
    qk_sb = [const.tile([128, 2, T], F32R, name=f"qk_{j}") for j in range(NPAIR)]
    v_sb = const.tile([128, TC, N_HEAD, HD + 1], F32R)  # +1 = ones column
    bqk_sb = const.tile([128, 12], F32)
    mb_sb = const.tile([128, TC], F32)
    bv_sb = const.tile([1, C], F32R)
    bp_sb = const.tile([1, C], F32R)
    ones_sb = const.tile([128, 128], F32R)
    wqk_sb: dict = {}

    def prefetch_wqk(m, eng=None, split=False):
        # 6-slot ring: the DMA into a reused slot waits (in-queue) for the
        # previous chunk's matmuls, which throttles prefetch automatically.
        # split=True lands ko chunks individually so the first ldweights can
        # start as soon as possible (prologue-critical chunks only).
        t = wqk_pool.tile([128, KO, 128], BF16, tag="wqk", name=f"wqk_{m}")
        if split:
            for ko in range(KO):
                (eng or nc.sync).dma_start(t[:, ko], wqk_r[m][:, ko])
        else:
            (eng or nc.sync).dma_start(t[:], wqk_r[m])
        wqk_sb[m] = t

    # ---- input DMAs --------------------------------------------------
    # sync queue: x only (the compute-critical stream); vector queue: W_v;
    # gpsimd queue: small tensors first, the fat W_proj last. The v ones
    # column is memset on-engine (a strided DMA here took ~30us and jammed
    # the queue behind it).
    xT_r = xT_d.rearrange("(ko p) t -> p ko t", p=128)
    wv_r = wa_d[:, 2 * C : 3 * C].rearrange("(ko p) n -> p ko n", p=128)
    wqk_r = [
        wa_d[:, m * 128 : (m + 1) * 128].rearrange("(ko p) n -> p ko n", p=128)
        for m in range(12)
    ]
    prefetch_wqk(0, nc.gpsimd)
    prefetch_wqk(6, nc.gpsimd)
    nc.scalar.dma_start(ones_sb[:], ones_d)
    nc.scalar.dma_start(bv_sb[:], bv_d)
    nc.scalar.dma_start(bqk_sb[:], bqk_d)
    nc.scalar.dma_start(mb_sb[:], mb_d)
    nc.scalar.dma_start(bp_sb[:], bp_d)
    for ko in range(KO):
        nc.sync.dma_start(xT_sb[ko][:], xT_r[:, ko])
        nc.scalar.dma_start(wv_sb[ko][:], wv_r[:, ko])
    # remaining qk weights, in consumption order
    for j in range(1, NPAIR):
        prefetch_wqk(j)
        prefetch_wqk(6 + j)
    nc.scalar.dma_start(wp_sb[:], wp_d.rearrange("(ko p) n -> p ko n", p=128))
    nc.vector.tensor_copy(
        out=v_sb[:, :, :, HD],
        in_=ones_sb[:, 0 : TC * N_HEAD].rearrange("p (a b) -> p a b", b=N_HEAD),
    )
    # warm the Exp activation table while the prologue DMAs stream
    warm_sb = const.tile([1, 2], F32, name="warm")
    nc.vector.memset(warm_sb[:], 0.0)
    nc.scalar.activation(warm_sb[0:1, 0:1], warm_sb[0:1, 1:2], AF.Exp)

    segs = [(0, 512), (512, 256)]

    # ---- fill-chunk generators (yield once per PE bundle) ------------
    def gen_v_chunk(tcc):
        # one PSUM tile at a time so two chunks can pipeline in the 2-slot
        # fill pool (a chunk holding both slots serializes chunk boundaries)
        for j, (off, w) in enumerate(segs):
            ps = fl_ps.tile([128, 512], F32, tag="fl", name=f"ps1b_{tcc}_{j}")
            nc.tensor.matmul(
                ps[:, :w],
                ones_sb[0:1, 0:128],
                bv_sb[0:1, off : off + w],
                start=True,
                stop=False,
            )
            for ko in range(KO):
                nc.tensor.matmul(
                    ps[:, :w],
                    xT_sb[ko][:, tcc * 128 : (tcc + 1) * 128],
                    wv_sb[ko][:, off : off + w],
                    start=False,
                    stop=(ko == KO - 1),
                )
                if ko in (1, 3):
                    yield
            nc.vector.tensor_copy(
                out=v_sb[:, tcc, off // HD : (off + w) // HD, 0:HD],
                in_=ps[:, :w].rearrange("p (h d) -> p h d", d=HD),
            )
            if j == 0:
                yield

    def gen_qk_chunk(j, half, m):
        # half 0 -> qT chunk (m = j), half 1 -> kT chunk (m = 6 + j)
        wqk = wqk_sb[m]
        pss = [
            fl_ps.tile([128, 512], F32, tag="fl", name=f"ps1a_{m}_{i}")
            for i in range(QN)
        ]
        for ko in range(KO):
            for nq in range(QN):
                nc.tensor.matmul(
                    pss[nq],
                    wqk[:, ko, :],
                    xT_sb[ko][:, nq * 512 : (nq + 1) * 512],
                    start=(ko == 0),
                    stop=(ko == KO - 1),
                )
            if ko < KO - 1:
                yield
        # psum -> SBUF with per-partition bias add (b_attn) on DVE
        for nq in range(QN):
            nc.vector.tensor_tensor(
                qk_sb[j][:, half, nq * 512 : (nq + 1) * 512],
                pss[nq],
                bqk_sb[:, m : m + 1].to_broadcast((128, 512)),
                mybir.AluOpType.add,
            )

    def gen_proj_chunk(tcc):
        o_sb = out_pool.tile([128, C], F32, tag="out", name=f"o_{tcc}")
        for j, (off, w) in enumerate(segs):
            ps = fl_ps.tile([128, 512], F32, tag="fl", name=f"ps4_{tcc}_{j}")
            nc.tensor.matmul(
                ps[:, :w],
                ones_sb[0:1, 0:128],
                bp_sb[0:1, off : off + w],
                start=True,
                stop=False,
            )
            for ko in range(KO):
                nc.tensor.matmul(
                    ps[:, :w],
                    qk_sb[ko][:, 0, tcc * 128 : (tcc + 1) * 128],
                    wp_sb[:, ko, off : off + w],
                    start=False,
                    stop=(ko == KO - 1),
                )
                if ko in (1, 3):
                    yield
            nc.scalar.activation(o_sb[:, off : off + w], ps[:, :w], AF.Copy)
            if j == 0:
                yield
        nc.sync.dma_start(y_d[tcc * 128 : (tcc + 1) * 128, :], o_sb[:])

    # ---- fill queue machinery ----------------------------------------
    fills: deque = deque()
    done: dict = {}

    def add_fill(name, gen):
        def wrapped():
            yield from gen
            done[name] = True
        done[name] = False
        fills.append(wrapped())

    def pump(n=1):
        k = 0
        while fills and k < n:
            try:
                next(fills[0])
                k += 1
            except StopIteration:
                fills.popleft()

    def drain(name):
        while not done[name]:
            if not fills:
                raise RuntimeError(f"drain({name}): queue empty")
            try:
                next(fills[0])
            except StopIteration:
                fills.popleft()

    # ---- attention for one head pair ---------------------------------
    pending_norm = [None]

    def emit_pair_attention(j, qc, pump_n=None):
        if True:
            qk = qk_sb[j]
            qsl = slice(qc * 512, (qc + 1) * 512)
            ava = av_ps.tile([65, 512], F32, tag="av", name=f"ava_{j}_{qc}")
            avb = av_ps.tile([65, 512], F32, tag="av", name=f"avb_{j}_{qc}")

            def emit_av(kc, e):
                if j == 0:
                    drain(f"v{kc}")
                nc.tensor.matmul(
                    ava, v_sb[:, kc, 2 * j, :], e[:, 0:512],
                    start=(kc == 0), stop=(kc == TC - 1),
                )
                nc.tensor.matmul(
                    avb, v_sb[:, kc, 2 * j + 1, :], e[:, 512:1024],
                    start=(kc == 0), stop=(kc == TC - 1),
                )

            prev = None
            for kc in range(TC):
                ksl = slice(kc * 128, (kc + 1) * 128)
                sc = sc_ps.tile([128, 1024], F32, tag="sc", name=f"sc_{j}_{qc}_{kc}")
                # head a (partitions 0-63) and head b (64-127)
                nc.tensor.matmul(
                    sc[:, 0:512], qk[0:64, 1, ksl], qk[0:64, 0, qsl],
                    start=True, stop=True,
                )
                nc.tensor.matmul(
                    sc[:, 512:1024], qk[64:128, 1, ksl], qk[64:128, 0, qsl],
                    start=True, stop=True,
                )
                e = e_pool.tile([128, 1024], F32R, tag="e", name=f"e_{j}_{qc}_{kc}")
                nc.scalar.activation(
                    e, sc, AF.Exp, bias=mb_sb[:, kc : kc + 1], scale=0.125
                )
                # issue the PREVIOUS iteration's AV matmuls after the next
                # score matmuls, with one fill bundle covering the exp wait
                if kc == 1 and pending_norm[0] is not None:
                    pending_norm[0]()
                    pending_norm[0] = None
                if prev is not None:
                    if not (j == 0 and qc == 0):
                        pump(pump_n if pump_n is not None else 1)
                    emit_av(*prev)
                prev = (kc, e)
            emit_av(*prev)

            # normalize — emission DEFERRED until after the NEXT sweep's
            # first scores: the concat-shift DMA below is tracked at tile
            # granularity, so emitting it before the next sweep's reads of
            # qk_sb would make those scores falsely wait out the whole
            # ~2-3us DRAM-bounce chain at every sweep boundary.
            def do_norm(j=j, qc=qc, qk=qk, qsl=qsl, ava=ava, avb=avb):
                # evacuate the AV accumulators first so the PSUM slots free
                r_t = r_pool.tile([65, 1024], F32, tag="r", name=f"r_{j}_{qc}")
                t_sb = tmp_pool.tile([64, 512], F32R, tag="tmp", name=f"tmp_{j}_{qc}")
                nc.vector.tensor_copy(out=r_t[64:65, 0:512], in_=ava[64:65, :])
                nc.vector.tensor_copy(out=r_t[64:65, 512:1024], in_=avb[64:65, :])
                nc.vector.tensor_copy(out=qk[0:64, 0, qsl], in_=ava[0:64, :])
                nc.vector.tensor_copy(out=t_sb[:], in_=avb[0:64, :])
                # denominators -> DRAM-bounce partition broadcast -> fast
                # reciprocal -> multiply in place
                rd = rd_pool.tile([1, 1024], F32, tag="rd", name=f"rd_{j}_{qc}")
                nc.gpsimd.dma_start(rd[:], r_t[64:65, :])
                rbw = rb_pool.tile([64, 1024], F32, tag="rbw", name=f"rbw_{j}_{qc}")
                nc.gpsimd.dma_start(rbw[:], rd.to_broadcast((64, 1024)))
                rb = rb_pool.tile([64, 1024], F32, tag="rb", name=f"rb_{j}_{qc}")
                nc.vector.reciprocal_approx_fast(out=rb[:], in_=rbw[:])
                # head a -> concat partitions 0-63 (in place in the qT half)
                nc.vector.tensor_mul(
                    out=qk[0:64, 0, qsl], in0=qk[0:64, 0, qsl], in1=rb[:, 0:512]
                )
                # head b -> concat partitions 64-127 (SBUF->SBUF DMA shift)
                nc.vector.tensor_mul(out=t_sb[:], in0=t_sb[:], in1=rb[:, 512:1024])
                nc.gpsimd.dma_start(qk[64:128, 0, qsl], t_sb[:])

            pending_norm[0] = do_norm

    # ---- schedule (qc-major) -----------------------------------------
    # prologue: pair 0's q/k chunks inline, interleaved per-ko so both
    # consume each xT chunk as its DMA lands; everything else is fill work.
    # qc-major order lets the first-half projection chunks overlap the
    # whole second attention sweep instead of forming a serial tail.
    alive = [gen_qk_chunk(0, 0, 0), gen_qk_chunk(0, 1, 6)]
    while alive:
        for gen in list(alive):
            try:
                next(gen)
            except StopIteration:
                alive.remove(gen)
    for tcc in range(TC):
        add_fill(f"v{tcc}", gen_v_chunk(tcc))
    for j in range(1, NPAIR):
        add_fill(f"q{j}", gen_qk_chunk(j, 0, j))
        add_fill(f"k{j}", gen_qk_chunk(j, 1, 6 + j))

    # j-major sweep with a precisely rationed fill diet: gapless PE needs
    # ~262ns of fill per iteration (Act exp 1115ns minus scores+AV 853ns);
    # alternating pump(1)/pump(2) of 512-col half-bundles averages ~320ns,
    # making the qk prefetch supply last through all six pairs.
    for j in range(NPAIR):
        if j > 0:
            drain(f"q{j}")
            drain(f"k{j}")
        emit_pair_attention(j, 0)
        if j == NPAIR - 1:
            # every pair has written its first-half concat: the first four
            # projection chunks can fill pair 5's second sweep
            for tcc in range(4):
                add_fill(f"p{tcc}", gen_proj_chunk(tcc))
        emit_pair_attention(j, 1, pump_n=2 if j == NPAIR - 1 else None)

    # drain the leftover first-half projection pops BEFORE emitting the
    # last sweep's norm — emitted after it they would falsely wait out the
    # whole DRAM-bounce chain (tile-granular DMA tracking); emitted before,
    # they cover the chain's ~5us latency with real PE work
    while fills:
        pump(4)
    # the last sweep's norm must precede the second-half projection (its
    # ko=5 matmuls genuinely consume pair 5's qc1 concat)
    pending_norm[0]()
    pending_norm[0] = None
    for tcc in range(4, TC):
        add_fill(f"p{tcc}", gen_proj_chunk(tcc))
    while fills:
        pump(4)


def _get_program():
    if "nc" in _cache:
        return _cache["nc"]
    nc = bacc.Bacc(
        "TRN2", target_bir_lowering=False, debug=False, enable_asserts=True
    )
    aps = {
        "xT": nc.dram_tensor("xT", [C, T], BF16, kind="ExternalInput").ap(),
        "Wa": nc.dram_tensor("Wa", [C, 3 * C], BF16, kind="ExternalInput").ap(),
        "Wp": nc.dram_tensor("Wp", [C, C], F32R, kind="ExternalInput").ap(),
        "bqk": nc.dram_tensor("bqk", [128, 12], F32, kind="ExternalInput").ap(),
        "bv": nc.dram_tensor("bv", [1, C], F32R, kind="ExternalInput").ap(),
        "bp": nc.dram_tensor("bp", [1, C], F32R, kind="ExternalInput").ap(),
        "mb": nc.dram_tensor("mb", [128, TC], F32, kind="ExternalInput").ap(),
        "y": nc.dram_tensor("y", [T, C], F32, kind="ExternalOutput").ap(),
        "ones": nc.dram_tensor("ones", [128, 128], F32R, kind="ExternalInput").ap(),
    }
    with tile.TileContext(nc) as tc_ctx, ExitStack() as ctx:
        aps["ctx"] = ctx
        _emit_kernel(tc_ctx, aps)
    nc.compile()
    _cache["nc"] = nc
    return nc


def _make_in_maps(inputs):
    x = np.asarray(inputs["x"], np.float32)
    mask = np.asarray(inputs["attn_mask"])
    Wa = np.asarray(inputs["W_attn"], np.float32)
    ba = np.asarray(inputs["b_attn"], np.float32)
    Wp = np.asarray(inputs["W_proj"], np.float32)
    bp = np.asarray(inputs["b_proj"], np.float32)

    bqk = np.ascontiguousarray(ba[: 2 * C].reshape(12, 128).T)
    bv = np.ascontiguousarray(ba[2 * C :].reshape(1, C))
    bpr = np.ascontiguousarray(bp.reshape(1, C))
    Wab = np.ascontiguousarray(Wa.astype(ml_dtypes.bfloat16))
    Wpb = np.ascontiguousarray(Wp)
    in_maps = []
    for b in range(B):
        mb = np.where(mask[b] == 0, np.float32(-30.0), np.float32(0.0))
        mb = np.ascontiguousarray(mb.astype(np.float32).reshape(TC, 128).T)
        in_maps.append(
            {
                "xT": np.ascontiguousarray(x[b].T.astype(ml_dtypes.bfloat16)),
                "Wa": Wab,
                "Wp": Wpb,
                "bqk": bqk,
                "bv": bv,
                "bp": bpr,
                "mb": mb,
                "ones": _ONES,
            }
        )
    return in_maps


def _run(inputs, trace=False):
    nc = _get_program()
    in_maps = _make_in_maps(inputs)
    res = bass_utils.run_bass_kernel_spmd(
        nc, in_maps, core_ids=list(range(B)), trace=trace
    )
    y = np.stack([res.results[b]["y"] for b in range(B)], axis=0)
    return y, res


def kernel(**inputs) -> np.ndarray:
    y, _ = _run(inputs, trace=False)
    return y
